# revision 32
# baseline (speedup 1.0000x reference)
"""Dilated block attention + output projection on 8 trn2 cores.

Sharding: core c handles batch b = c//2 and heads h = 4*(c%2) .. +3.
Each core computes the full dilated-attention combine for its 4 (b,h)
pairs and a partial output projection (contraction over its 4 heads'
256 hidden dims).  The host sums the two half-hidden partials per batch
and adds the bias.

Math note: the reference's stabilized-softmax + detached-expsum
reweighting collapses to the unstabilized form
    out[p] = (sum_d exp(S_d) @ V_d  scattered to p) / (sum_d rowsum exp(S_d))
which is what the kernel computes (scores ~ N(0,1), no overflow risk).

Device layout per (b,h), per dilation branch: the host packs ONE blob
[128, W] per branch holding, in SBUF layout:
  - Q^T [64, Ld] duplicated onto both partition halves (matmul rhs for
    both PE row groups),
  - K^T k-tiles parity-split: even k-tiles on partitions 0-63, odd on
    64-127 (so consecutive K=64 QK matmuls land on different PE row
    groups and run concurrently),
  - V k-tile slabs [128, 65] with a ones column (PV matmul with M=65
    gives the exp row-sum on psum row 64 for free).
One DMA per branch.  S^T = matmul(lhsT=K^T[64,128], rhs=Q^T[64,512]) to
PSUM; exp on ScalarE (PSUM->SBUF, scale=0.125 folds 1/sqrt(hd)); PV
accumulates over k-tiles into a [65, 512] psum window; DVE scatter-adds
windows into per-head accumulators [65, 4096]; 1/w via custom-DVE fast
reciprocal + K=1 ones-matmul partition broadcast; o_proj as 4
accumulating K=64 matmuls per M-tile against Wo^T slices.

Matmul operands are bf16 (fp32 matmuls run as two PE passes on trn2);
psum accumulation and the softmax combine stay fp32.  The PE stream is
software-pipelined (QK/exp of group i+1 issued before PV of group i) so
the in-order PE queue never head-of-line blocks on the ScalarE exp.
"""

import ml_dtypes
import numpy as np

BF16_NP = ml_dtypes.bfloat16

B, H, L, HD = 4, 8, 4096, 64
HIDDEN = H * HD
DILS = (1, 2, 4, 8)
BLOCK = 1024
PB = 4  # (b,h) pairs per core
NCORES = 8
LDS = [L // d for d in DILS]  # 4096, 2048, 1024, 512
NKTS = [ld // 128 for ld in LDS]  # 32, 16, 8, 4
# blob widths per branch: Q dup (Ld) + K parity-split (Ld/2) + V slabs (nkt*65)
WS = [ld + ld // 2 + nkt * 65 for ld, nkt in zip(LDS, NKTS)]
BOFFS = [sum(WS[:i]) for i in range(len(WS))]
WSUM = sum(WS)
QCH = 512  # q-chunk (strided-domain positions) per psum window

# Schraudolph fast-exp on the DVE for a fraction of the exp work (the
# ScalarE ACTIVATE stream is the kernel bottleneck): bf16 bit pattern of
# exp(s/8) ~= round(s * 128*log2(e)/8 + 128*(127 - C)) written as int16
# into a bf16 tile via AP bitcast.  One 1x tensor_scalar per tile vs the
# 1x ACTIVATE -- splitting ~3/8 of tiles to the DVE balances the engines.
SCH_C = 0.0437
SCH_A = float(128.0 * np.log2(np.e) / 8.0)
SCH_B = float(128.0 * (127.0 - SCH_C))
# job j uses the DVE-exp when DVE_PAT[j % len] is 1 (3/8 duty)
DVE_PAT = (1, 0, 0, 1, 0, 0, 1, 0)

_PROGRAM = None


def build_program():
    """Build the (SPMD, identical on all cores) Bass program."""
    from contextlib import ExitStack

    import concourse.tile as tile
    from concourse import bacc, mybir

    F32 = mybir.dt.float32
    BF16 = mybir.dt.bfloat16
    I16 = mybir.dt.int16
    nc = bacc.Bacc("TRN2", target_bir_lowering=False, debug=False)

    blob_d = nc.dram_tensor("blob", [PB, 128, WSUM], BF16, kind="ExternalInput")
    # head-pair-stacked Wo^T slices: wot[p, k<64] = head 2p dim k,
    # wot[p, k>=64] = head 2p+1 dim k-64 (K=128 o_proj contraction)
    wot_d = nc.dram_tensor("wot", [PB // 2, 128, HIDDEN], BF16, kind="ExternalInput")
    # per-head-pair o_proj partials; the host sums them (pair 0's pass
    # overlaps heads 2-3 on-device instead of serializing at the end)
    out_d = nc.dram_tensor("out", [PB // 2, L, HIDDEN], F32, kind="ExternalOutput")

    with tile.TileContext(nc) as tc, ExitStack() as ctx:
        consts = ctx.enter_context(tc.tile_pool(name="consts", bufs=1))
        br_pool = ctx.enter_context(tc.tile_pool(name="br", bufs=1))
        e_pool = ctx.enter_context(tc.tile_pool(name="ep", bufs=5))
        acc_pool = ctx.enter_context(tc.tile_pool(name="accp", bufs=1))
        io_pool = ctx.enter_context(tc.tile_pool(name="iop", bufs=2))
        st_psum = ctx.enter_context(tc.tile_pool(name="stp", bufs=2, space="PSUM"))
        pv_psum = ctx.enter_context(tc.tile_pool(name="pvp", bufs=2, space="PSUM"))

        zero_bias = consts.tile([128, 1], F32, tag="zb")
        nc.vector.memset(zero_bias, 0.0)
        ones_row = consts.tile([1, 64], BF16, tag="ones_row")
        nc.vector.memset(ones_row, 1.0)

        wot_sb = consts.tile([128, PB // 2, HIDDEN], BF16, tag="wot")
        nc.sync.dma_start(out=wot_sb, in_=wot_d.rearrange("j r c -> r j c"))

        # o_proj lhsT: head pair p stacked on partitions [0:64] / [64:128]
        oacc_tiles = [
            acc_pool.tile([128, L], BF16, tag=f"oacc{p}", bufs=1, name=f"oacc{p}")
            for p in range(PB // 2)
        ]

        from collections import deque

        exp_ctr = [0]
        cp_ctr = [0]
        pending_su = deque()
        oproj_backlog = []
        NBLK = L // BLOCK  # 4 position blocks of 1024
        for j in range(PB):
            acc = acc_pool.tile([65, L], F32, tag="acc", bufs=2, name=f"acc{j}")
            bt_tiles = {}

            def get_bt(di):
                if di not in bt_tiles:
                    bt = br_pool.tile(
                        [128, WS[di]], BF16, tag=f"b{di}", bufs=2, name=f"bt{di}"
                    )
                    nc.sync.dma_start(
                        out=bt, in_=blob_d[j, :, BOFFS[di] : BOFFS[di] + WS[di]]
                    )
                    bt_tiles[di] = bt
                return bt_tiles[di]

            for di in (3, 2, 0, 1):
                get_bt(di)

            # Job list, position-major: every job is a (2-ktile, 512-q)
            # QK+exp unit.  d4/d8 jobs span several position blocks; their
            # E tiles live longer (bigger tag pools).
            jobs = []
            jm = {}

            def add_job(key, di, kts, q0, tag, bufs, blk_end=None):
                jb = dict(
                    di=di,
                    kts=kts,
                    q0=q0,
                    tag=tag,
                    bufs=bufs,
                    blk_end=blk_end,
                    dve_exp=bool(DVE_PAT[exp_ctr[0] % len(DVE_PAT)]),
                )
                exp_ctr[0] += 1
                jm[key] = jb
                jobs.append(jb)

            for g in range(2):  # d8: Ld=512, 4 ktiles
                add_job((3, g), 3, [2 * g, 2 * g + 1], 0, "e8", 4)
            for g in range(4):  # d4 first half (blocks 0-1)
                add_job((2, 0, g), 2, [2 * g, 2 * g + 1], 0, "e4", 10)
            for B in range(NBLK):
                if B == 2:
                    for g in range(4):  # d4 second half (blocks 2-3)
                        add_job((2, 1, g), 2, [2 * g, 2 * g + 1], 512, "e4", 10)
                for qc in range(2):
                    for g in range(4):  # d1 block B
                        add_job(
                            (0, B, qc, g),
                            0,
                            [B * 8 + 2 * g, B * 8 + 2 * g + 1],
                            B * 1024 + qc * 512,
                            "et",
                            18,
                        )
                for g in range(4):  # d2 half-block (positions B*1024..+1024)
                    add_job(
                        (1, B, g),
                        1,
                        [(B // 2) * 8 + 2 * g, (B // 2) * 8 + 2 * g + 1],
                        B * 512,
                        "et",
                        18,
                        blk_end=B if g == 3 else None,
                    )

            def emit_qk_exp(job):
                """QK matmuls for the 2-ktile group -> exp to a bf16 E tile."""
                di, q0 = job["di"], job["q0"]
                Ld = LDS[di]
                bt = get_bt(di)
                st = st_psum.tile([128, 2, QCH], F32, tag="st", name="st")
                for i, tg in enumerate(job["kts"]):
                    half = tg % 2
                    k0 = Ld + (tg // 2) * 128
                    nc.tensor.matmul(
                        st[:, i, :],
                        bt[half * 64 : (half + 1) * 64, k0 : k0 + 128],
                        bt[half * 64 : (half + 1) * 64, q0 : q0 + QCH],
                        start=True,
                        stop=True,
                    )
                et = e_pool.tile(
                    [128, 2, QCH], BF16, tag=job["tag"], bufs=job["bufs"], name="et"
                )
                if job["dve_exp"]:
                    nc.vector.tensor_scalar(
                        et.bitcast(I16),
                        st,
                        SCH_A,
                        SCH_B,
                        mybir.AluOpType.mult,
                        mybir.AluOpType.add,
                    )
                else:
                    nc.scalar.activation(
                        et,
                        st,
                        mybir.ActivationFunctionType.Exp,
                        bias=zero_bias,
                        scale=0.125,
                    )
                job["et"] = et

            # Window = 512 output positions.  All four branches' PV matmuls
            # accumulate into one [65, 512] psum window (strided column APs
            # scatter d>1 branches); psum row 64 collects the total exp-sum
            # via the V ones column.  start=True only on the first (dense
            # d=1) matmul; later strided matmuls accumulate.
            def queue_window(B, acc=None, jm=None, bts=None):
                # One [128, 1024] psum tile covers BOTH 512-windows of
                # position block B, so every d2/d4/d8 k-tile is a single
                # strided matmul across the whole block (36 MMs + 1 copy
                # per block instead of 56 + 2).  Everything is bound
                # eagerly -- these closures run deferred, possibly during
                # the NEXT head's job stream.
                acc_ = acc
                d1j = {
                    (qc, g): jm[(0, B, qc, g)] for qc in range(2) for g in range(4)
                }
                d2j = [jm[(1, B, g)] for g in range(4)]
                d4j = [jm[(2, B // 2, g)] for g in range(4)]
                d8j = [jm[(3, g)] for g in range(2)]
                pvt = [None]
                BW = 2 * QCH

                def su_d1(qc):
                    if qc == 0:
                        pvt[0] = pv_psum.tile([128, BW], F32, tag="pv", name="pv")
                    bt = bts[0]
                    vbase = LDS[0] + LDS[0] // 2
                    for g in range(4):
                        jb = d1j[(qc, g)]
                        for i, tg in enumerate(jb["kts"]):
                            nc.tensor.matmul(
                                pvt[0][0:65, qc * QCH : (qc + 1) * QCH],
                                bt[:, vbase + tg * 65 : vbase + tg * 65 + 65],
                                jb["et"][:, i, :],
                                start=(g == 0 and i == 0),
                                stop=False,
                                skip_group_check=True,
                            )

                def su_d2():
                    # emitted LAST (freshest E tiles -> most exp lead time);
                    # carries the stop and the block close
                    bt = bts[1]
                    vbase = LDS[1] + LDS[1] // 2
                    for g in range(4):
                        jb = d2j[g]
                        for i, tg in enumerate(jb["kts"]):
                            nc.tensor.matmul(
                                pvt[0][0:65, 0 : BW : 2],
                                bt[:, vbase + tg * 65 : vbase + tg * 65 + 65],
                                jb["et"][:, i, :],
                                start=False,
                                stop=(g == 3 and i == 1),
                                skip_group_check=True,
                            )
                    # close: copy the combined block (incl. w row) to acc,
                    # alternating the copy engine to balance ACT/DVE load
                    dst = acc_[:, B * BW : (B + 1) * BW]
                    if cp_ctr[0] % 2 == 0:
                        nc.scalar.copy(out=dst, in_=pvt[0][0:65, :])
                    else:
                        nc.vector.tensor_copy(out=dst, in_=pvt[0][0:65, :])
                    cp_ctr[0] += 1

                def su_d4():
                    bt = bts[2]
                    vbase = LDS[2] + LDS[2] // 2
                    off = (B % 2) * 256
                    for g in range(4):
                        jb = d4j[g]
                        for i, tg in enumerate(jb["kts"]):
                            nc.tensor.matmul(
                                pvt[0][0:65, 0 : BW : 4],
                                bt[:, vbase + tg * 65 : vbase + tg * 65 + 65],
                                jb["et"][:, i, off : off + 256],
                                start=False,
                                stop=False,
                                skip_group_check=True,
                            )

                def su_d8():
                    bt = bts[3]
                    vbase = LDS[3] + LDS[3] // 2
                    off = B * 128
                    for g in range(2):
                        jb = d8j[g]
                        for i, tg in enumerate(jb["kts"]):
                            nc.tensor.matmul(
                                pvt[0][0:65, 0 : BW : 8],
                                bt[:, vbase + tg * 65 : vbase + tg * 65 + 65],
                                jb["et"][:, i, off : off + 128],
                                start=False,
                                stop=False,
                                skip_group_check=True,
                            )

                return [
                    lambda: su_d1(0),
                    lambda: su_d1(1),
                    su_d4,
                    su_d8,
                    su_d2,
                ]

            # Pump window subunits with a lag (backlog kept >4) so their E
            # tiles are several jobs old by emission -- the in-order PE
            # queue then never stalls on a late exp.  One deferred-tail
            # closure per job keeps the previous head's normalize moving.
            for job in jobs:
                emit_qk_exp(job)
                if len(pending_su) > 4:
                    pending_su.popleft()()
                if len(pending_su) > 10:
                    pending_su.popleft()()
                if job["blk_end"] is not None:
                    B = job["blk_end"]
                    while len(pending_su) > 2:
                        pending_su.popleft()()
                    bts = [bt_tiles[di] for di in range(4)]
                    pending_su.extend(queue_window(B, acc, jm, bts))
                    if oproj_backlog:
                        pending_su.append(oproj_backlog.pop(0))
                    if oproj_backlog:
                        pending_su.append(oproj_backlog.pop(0))

            # normalize tail: oacc = acc[0:64, :] * (1 / acc[64, :]) in
            # bf16, deferred into the next head's job stream (the 1/w DMA
            # roundtrip would otherwise idle the PE at head boundaries).
            def make_tail(j=j, acc=acc):
                st8 = {}

                def t_recip():
                    # w row [1, L] reshaped to [128, 32] by DMA so the
                    # reciprocal + cast run on 128 DVE lanes instead of 1
                    wrs = io_pool.tile([128, L // 128], F32, tag="wrs", bufs=2, name="wrs")
                    nc.sync.dma_start(out=wrs, in_=acc[64:65, :])
                    nc.vector.reciprocal_approx_fast(out=wrs, in_=wrs)
                    wrsb = io_pool.tile([128, L // 128], BF16, tag="wrsb", bufs=2, name="wrsb")
                    nc.vector.tensor_copy(out=wrsb, in_=wrs)
                    wrowb = io_pool.tile([1, L], BF16, tag="wrowb", bufs=2, name="wrowb")
                    nc.sync.dma_start(out=wrowb, in_=wrsb)
                    st8["wrowb"] = wrowb
                    if j % 2 == 0:
                        st8["odst"] = oacc_tiles[j // 2]
                    else:
                        st8["odst"] = io_pool.tile(
                            [64, L], BF16, tag="oscr", bufs=1, name="oscr"
                        )

                def t_bc(w):
                    ws = slice(w * QCH, (w + 1) * QCH)
                    # broadcast 1/w across 64 partitions via a K=1 ones-MM
                    bc = pv_psum.tile([64, QCH], F32, tag="pv", name="bc")
                    nc.tensor.matmul(
                        bc,
                        ones_row[0:1, :],
                        st8["wrowb"][0:1, ws],
                        start=True,
                        stop=True,
                    )
                    nc.vector.tensor_mul(
                        out=st8["odst"][0:64, ws], in0=acc[0:64, ws], in1=bc
                    )

                def t_fin():
                    if j % 2 == 1:
                        # DVE lanes are partition-locked; a SBUF->SBUF DMA
                        # moves the odd head onto partitions 64..127 of the
                        # pair-stacked o_proj lhsT
                        nc.sync.dma_start(
                            out=oacc_tiles[j // 2][64:128, :], in_=st8["odst"]
                        )

                return (
                    [t_recip]
                    + [lambda w=w: t_bc(w) for w in range(L // QCH)]
                    + [t_fin]
                )

            # the tail goes on the SAME fifo, behind this head's last
            # window closes -- its acc w-row read must not be emitted
            # before the window copies it depends on
            pending_su.extend(make_tail())

            # per-pair o_proj partial: out[p] = oaccP_p^T @ wot_p (K=128).
            # Pair 0's pass is chunked into the fifo after head 1 so it
            # overlaps heads 2-3 instead of serializing at the very end.
            def make_oproj(p):
                chunks = []
                for mtc in range(0, L // 128, 2):

                    def chunk(mtc=mtc, p=p):
                        # two m-tiles per psum tile -> one copy + one DMA
                        po = pv_psum.tile(
                            [128, 2, HIDDEN], F32, tag="pv", name="po"
                        )
                        for k in range(2):
                            mt = mtc + k
                            nc.tensor.matmul(
                                po[:, k, :],
                                oacc_tiles[p][:, mt * 128 : (mt + 1) * 128],
                                wot_sb[:, p, :],
                                start=True,
                                stop=True,
                                skip_group_check=True,
                            )
                        ot = io_pool.tile(
                            [128, 2, HIDDEN], F32, tag="ot", name="ot"
                        )
                        if cp_ctr[0] % 2 == 0:
                            nc.scalar.copy(out=ot, in_=po)
                        else:
                            nc.vector.tensor_copy(out=ot, in_=po)
                        cp_ctr[0] += 1
                        od = out_d[
                            p, mtc * 128 : (mtc + 2) * 128, :
                        ].rearrange("(g r) c -> r g c", g=2)
                        nc.sync.dma_start(out=od, in_=ot)

                    chunks.append(chunk)
                return chunks

            if j == 1:
                oproj_backlog.extend(make_oproj(0))
            if j == PB - 1:
                pending_su.extend(make_oproj(1))

        while pending_su:
            pending_su.popleft()()

    nc.compile()
    return nc


def get_program():
    global _PROGRAM
    if _PROGRAM is None:
        _PROGRAM = build_program()
    return _PROGRAM


def _branch_blob(qT, kT, vv, di):
    """Pack one dilation branch into the [128, W] SBUF-layout blob.

    qT, kT: [64, Ld] transposed Q/K for this branch; vv: [Ld, 65] V plus
    ones column."""
    Ld, nkt = LDS[di], NKTS[di]
    q_part = np.concatenate([qT, qT], axis=0)  # [128, Ld]
    k3 = kT.reshape(64, nkt, 128)
    k_part = np.concatenate(
        [
            k3[:, 0::2, :].reshape(64, -1),
            k3[:, 1::2, :].reshape(64, -1),
        ],
        axis=0,
    )  # [128, Ld/2]
    v_part = vv.reshape(nkt, 128, 65).transpose(1, 0, 2).reshape(128, nkt * 65)
    return np.concatenate([q_part, k_part, v_part], axis=1)


def make_in_maps(query_states, key_states, value_states, Wo):
    q = np.asarray(query_states, dtype=np.float32)
    k = np.asarray(key_states, dtype=np.float32)
    v = np.asarray(value_states, dtype=np.float32)
    Wo = np.asarray(Wo, dtype=np.float32)

    in_maps = []
    for c in range(NCORES):
        b, hs = c // 2, (c % 2) * PB
        blob = np.empty((PB, 128, WSUM), BF16_NP)
        wot = np.empty((PB // 2, 128, HIDDEN), BF16_NP)
        for j in range(PB):
            h = hs + j
            for di, d in enumerate(DILS):
                Ld = LDS[di]
                vv = np.empty((Ld, 65), np.float32)
                vv[:, 0:64] = v[b, h, ::d, :]
                vv[:, 64] = 1.0
                blob[j, :, BOFFS[di] : BOFFS[di] + WS[di]] = _branch_blob(
                    np.ascontiguousarray(q[b, h, ::d, :].T),
                    np.ascontiguousarray(k[b, h, ::d, :].T),
                    vv,
                    di,
                )
            # head-pair-stacked o_proj weights (K=128 contraction)
            wot[j // 2, (j % 2) * 64 : (j % 2 + 1) * 64, :] = Wo[
                :, h * 64 : (h + 1) * 64
            ].T
        in_maps.append({"blob": blob, "wot": wot})
    return in_maps


def combine_outputs(results, bo):
    bo = np.asarray(bo, dtype=np.float32)
    out = np.empty((B, L, HIDDEN), np.float32)
    for b in range(B):
        out[b] = (
            results[2 * b]["out"].sum(0)
            + results[2 * b + 1]["out"].sum(0)
            + bo
        )
    return out


def kernel(
    query_states,
    key_states,
    value_states,
    Wo,
    bo,
    _trace=False,
    _tmpdir=None,
    _results=[None],
):
    from concourse.bass_utils import run_bass_kernel_spmd

    nc = get_program()
    in_maps = make_in_maps(query_states, key_states, value_states, Wo)
    res = run_bass_kernel_spmd(
        nc, in_maps, list(range(NCORES)), trace=_trace, tmpdir=_tmpdir
    )
    _results[0] = res
    return combine_outputs(res.results, bo)



# revision 42
# speedup vs baseline: 1.1022x; 1.1022x over previous
"""Dilated block attention + output projection on 8 trn2 cores.

Sharding: core c handles batch b = c//2 and heads h = 4*(c%2) .. +3.
Each core computes the full dilated-attention combine for its 4 (b,h)
pairs and a partial output projection (contraction over its 4 heads'
256 hidden dims).  The host sums the two half-hidden partials per batch
and adds the bias.

Math note: the reference's stabilized-softmax + detached-expsum
reweighting collapses to the unstabilized form
    out[p] = (sum_d exp(S_d) @ V_d  scattered to p) / (sum_d rowsum exp(S_d))
which is what the kernel computes (scores ~ N(0,1), no overflow risk).

Device layout per (b,h), per dilation branch: the host packs ONE blob
[128, W] per branch holding, in SBUF layout:
  - Q^T [64, Ld] duplicated onto both partition halves (matmul rhs for
    both PE row groups),
  - K^T k-tiles parity-split: even k-tiles on partitions 0-63, odd on
    64-127 (so consecutive K=64 QK matmuls land on different PE row
    groups and run concurrently),
  - V k-tile slabs [128, 65] with a ones column (PV matmul with M=65
    gives the exp row-sum on psum row 64 for free).
One DMA per branch.  S^T = matmul(lhsT=K^T[64,128], rhs=Q^T[64,512]) to
PSUM; exp on ScalarE (PSUM->SBUF, scale=0.125 folds 1/sqrt(hd)); PV
accumulates over k-tiles into a [65, 512] psum window; DVE scatter-adds
windows into per-head accumulators [65, 4096]; 1/w via custom-DVE fast
reciprocal + K=1 ones-matmul partition broadcast; o_proj as 4
accumulating K=64 matmuls per M-tile against Wo^T slices.

Matmul operands are bf16 (fp32 matmuls run as two PE passes on trn2);
psum accumulation and the softmax combine stay fp32.  The PE stream is
software-pipelined (QK/exp of group i+1 issued before PV of group i) so
the in-order PE queue never head-of-line blocks on the ScalarE exp.
"""

import ml_dtypes
import numpy as np

BF16_NP = ml_dtypes.bfloat16

B, H, L, HD = 4, 8, 4096, 64
HIDDEN = H * HD
DILS = (1, 2, 4, 8)
BLOCK = 1024
PB = 4  # (b,h) pairs per core
NCORES = 8
LDS = [L // d for d in DILS]  # 4096, 2048, 1024, 512
NKTS = [ld // 128 for ld in LDS]  # 32, 16, 8, 4
# blob widths per branch: Q dup (Ld) + K parity-split (Ld/2) + V slabs (nkt*65)
WS = [ld + ld // 2 + nkt * 65 for ld, nkt in zip(LDS, NKTS)]
BOFFS = [sum(WS[:i]) for i in range(len(WS))]
WSUM = sum(WS)
QCH = 512  # q-chunk (strided-domain positions) per psum window

_PROGRAM = None


def build_program():
    """Build the (SPMD, identical on all cores) Bass program."""
    from contextlib import ExitStack

    import concourse.tile as tile
    from concourse import bacc, mybir

    F32 = mybir.dt.float32
    BF16 = mybir.dt.bfloat16
    nc = bacc.Bacc("TRN2", target_bir_lowering=False, debug=False)

    blob_d = nc.dram_tensor("blob", [PB, 128, WSUM], BF16, kind="ExternalInput")
    # head-pair-stacked Wo^T slices: wot[p, k<64] = head 2p dim k,
    # wot[p, k>=64] = head 2p+1 dim k-64 (K=128 o_proj contraction)
    wot_d = nc.dram_tensor("wot", [PB // 2, 128, HIDDEN], BF16, kind="ExternalInput")
    out_d = nc.dram_tensor("out", [L, HIDDEN], F32, kind="ExternalOutput")

    with tile.TileContext(nc) as tc, ExitStack() as ctx:
        consts = ctx.enter_context(tc.tile_pool(name="consts", bufs=1))
        br_pool = ctx.enter_context(tc.tile_pool(name="br", bufs=1))
        e_pool = ctx.enter_context(tc.tile_pool(name="ep", bufs=5))
        acc_pool = ctx.enter_context(tc.tile_pool(name="accp", bufs=1))
        io_pool = ctx.enter_context(tc.tile_pool(name="iop", bufs=2))
        # st tiles hold 3 k-tiles of scores (one ACTIVATE of N=1536 instead
        # of 1.5 of N=1024 -- the ~470ns per-call ScalarE overhead is the
        # top non-stream cost on the bottleneck engine).  2 bufs x 3 banks
        # keeps the same 6 in-flight score k-tiles as 3 bufs x 2 banks.
        st_psum = ctx.enter_context(tc.tile_pool(name="stp", bufs=2, space="PSUM"))
        pv_psum = ctx.enter_context(tc.tile_pool(name="pvp", bufs=2, space="PSUM"))

        zero_bias = consts.tile([128, 1], F32, tag="zb")
        nc.vector.memset(zero_bias, 0.0)
        ones_row = consts.tile([1, 64], BF16, tag="ones_row")
        nc.vector.memset(ones_row, 1.0)

        wot_sb = consts.tile([128, PB // 2, HIDDEN], BF16, tag="wot")
        nc.sync.dma_start(out=wot_sb, in_=wot_d.rearrange("j r c -> r j c"))

        acc_tiles = [
            acc_pool.tile([65, L], F32, tag=f"acc{j}", bufs=1, name=f"acc{j}")
            for j in range(PB)
        ]
        # o_proj lhsT: head pair p stacked on partitions [0:64] / [64:128]
        oacc_tiles = [
            acc_pool.tile([128, L], BF16, tag=f"oacc{p}", bufs=1, name=f"oacc{p}")
            for p in range(PB // 2)
        ]

        for j in range(PB):
            acc = acc_tiles[j]

            # Build the flat job list: one job per (branch, window, k-group).
            jobs = []
            bt_tiles = {}
            for di, d in enumerate(DILS):
                Ld = LDS[di]
                bs = min(BLOCK, Ld)
                nblk = Ld // bs
                nkt_blk = bs // 128
                for blk in range(nblk):
                    for qc in range(bs // QCH):
                        q0 = blk * bs + qc * QCH
                        kts = list(range(nkt_blk))
                        groups = [kts[x : x + 3] for x in range(0, nkt_blk, 3)]
                        for gi, g in enumerate(groups):
                            jobs.append(
                                dict(
                                    di=di,
                                    d=d,
                                    blk=blk,
                                    nkt_blk=nkt_blk,
                                    q0=q0,
                                    g=g,
                                    first=(gi == 0),
                                    last=(gi == len(groups) - 1),
                                    done0=sum(len(x) for x in groups[:gi]),
                                )
                            )

            def get_bt(di):
                if di not in bt_tiles:
                    bufs = 1 if di <= 1 else 2
                    bt = br_pool.tile(
                        [128, WS[di]], BF16, tag=f"b{di}", bufs=bufs, name=f"bt{di}"
                    )
                    nc.sync.dma_start(
                        out=bt, in_=blob_d[j, :, BOFFS[di] : BOFFS[di] + WS[di]]
                    )
                    bt_tiles[di] = bt
                return bt_tiles[di]

            # prefetch the first branches
            get_bt(0)
            get_bt(1)

            def emit_qk_exp(job):
                """QK matmuls for the group -> exp to a bf16 E tile."""
                di, q0, g = job["di"], job["q0"], job["g"]
                Ld = LDS[di]
                kbase = Ld
                bt = get_bt(di)
                gs = len(g)
                st = st_psum.tile([128, 3, QCH], F32, tag="st", name="st")
                for i, kt in enumerate(g):
                    tg = job["blk"] * job["nkt_blk"] + kt
                    half = tg % 2
                    k0 = kbase + (tg // 2) * 128
                    nc.tensor.matmul(
                        st[:, i, :],
                        bt[half * 64 : (half + 1) * 64, k0 : k0 + 128],
                        bt[half * 64 : (half + 1) * 64, q0 : q0 + QCH],
                        start=True,
                        stop=True,
                    )
                et = e_pool.tile([128, 3, QCH], BF16, tag="et", name="et")
                nc.scalar.activation(
                    et[:, 0:gs, :],
                    st[:, 0:gs, :],
                    mybir.ActivationFunctionType.Exp,
                    bias=zero_bias,
                    scale=0.125,
                )
                job["et"] = et

            def emit_pv(job):
                """PV accumulation for the group; combine if window done."""
                di, d = job["di"], job["d"]
                Ld = LDS[di]
                vbase = Ld + Ld // 2
                bt = get_bt(di)
                et = job["et"]
                pv = job["pv"]
                done = job["done0"]
                for i, kt in enumerate(job["g"]):
                    tg = job["blk"] * job["nkt_blk"] + kt
                    nc.tensor.matmul(
                        pv[0:65, :],
                        bt[:, vbase + tg * 65 : vbase + tg * 65 + 65],
                        et[:, i, :],
                        start=(done == 0),
                        stop=(done == job["nkt_blk"] - 1),
                        skip_group_check=True,
                    )
                    done += 1
                if job["last"]:
                    p0 = job["q0"] * d
                    if d == 1:
                        nc.vector.tensor_copy(
                            out=acc[:, p0 : p0 + QCH], in_=pv[0:65, :]
                        )
                    else:
                        dst = acc[:, p0 : p0 + QCH * d : d]
                        nc.vector.tensor_add(out=dst, in0=dst, in1=pv[0:65, :])

            # software pipeline, depth 2: QK/exp of job i, then PV of job
            # i-2, so the in-order PE queue never blocks on the ACT exp.
            from collections import deque

            pending = deque()
            cur_pv = None
            for idx, job in enumerate(jobs):
                if job["first"]:
                    cur_pv = pv_psum.tile([128, QCH], F32, tag="pv", name="pv")
                job["pv"] = cur_pv
                # prefetch next branch blob one branch ahead
                if idx > 0 and job["di"] != jobs[idx - 1]["di"] and job["di"] < 3:
                    get_bt(job["di"] + 1)
                emit_qk_exp(job)
                pending.append(job)
                if len(pending) > 3:
                    emit_pv(pending.popleft())
            while pending:
                emit_pv(pending.popleft())
            bt_tiles.clear()

            # normalize: oacc = acc[0:64, :] * (1 / acc[64, :]) in bf16.
            # The w row [1, L] is reshaped to [128, 32] by DMA so the
            # reciprocal + cast run on all 128 DVE lanes instead of one.
            wrs = io_pool.tile([128, L // 128], F32, tag="wrs", bufs=2)
            nc.sync.dma_start(out=wrs, in_=acc[64:65, :])
            nc.vector.reciprocal_approx_fast(out=wrs, in_=wrs)
            wrsb = io_pool.tile([128, L // 128], BF16, tag="wrsb", bufs=2)
            nc.vector.tensor_copy(out=wrsb, in_=wrs)
            wrowb = io_pool.tile([1, L], BF16, tag="wrowb", bufs=2)
            nc.sync.dma_start(out=wrowb, in_=wrsb)
            half = j % 2
            if half == 0:
                odst = oacc_tiles[j // 2]
            else:
                odst = io_pool.tile([64, L], BF16, tag="oscr", bufs=2)
            for w in range(L // QCH):
                ws = slice(w * QCH, (w + 1) * QCH)
                # broadcast 1/w across 64 partitions via a K=1 ones-matmul
                bc = pv_psum.tile([64, QCH], F32, tag="pv", name="bc")
                nc.tensor.matmul(
                    bc, ones_row[0:1, :], wrowb[0:1, ws], start=True, stop=True
                )
                nc.vector.tensor_mul(
                    out=odst[0:64, ws], in0=acc[0:64, ws], in1=bc
                )
            if half == 1:
                # DVE lanes are partition-locked; a SBUF->SBUF DMA moves the
                # odd head onto partitions 64..127 of the pair-stacked lhsT
                nc.sync.dma_start(out=oacc_tiles[j // 2][64:128, :], in_=odst)

        # partial o_proj: out[p, :] = sum_pairs oaccP[:, p]^T @ wot_p (K=128)
        for mt in range(L // 128):
            po = pv_psum.tile([128, HIDDEN], F32, tag="pv", name="po")
            for p in range(PB // 2):
                nc.tensor.matmul(
                    po,
                    oacc_tiles[p][:, mt * 128 : (mt + 1) * 128],
                    wot_sb[:, p, :],
                    start=(p == 0),
                    stop=(p == PB // 2 - 1),
                    skip_group_check=True,
                )
            ot = io_pool.tile([128, HIDDEN], F32, tag="ot")
            if mt % 2 == 0:
                nc.scalar.copy(out=ot, in_=po)
            else:
                nc.vector.tensor_copy(out=ot, in_=po)
            nc.sync.dma_start(out=out_d[mt * 128 : (mt + 1) * 128, :], in_=ot)

    nc.compile()
    return nc


def get_program():
    global _PROGRAM
    if _PROGRAM is None:
        _PROGRAM = build_program()
    return _PROGRAM


def _branch_blob(qT, kT, vv, di):
    """Pack one dilation branch into the [128, W] SBUF-layout blob.

    qT, kT: [64, Ld] transposed Q/K for this branch; vv: [Ld, 65] V plus
    ones column."""
    Ld, nkt = LDS[di], NKTS[di]
    q_part = np.concatenate([qT, qT], axis=0)  # [128, Ld]
    k3 = kT.reshape(64, nkt, 128)
    k_part = np.concatenate(
        [
            k3[:, 0::2, :].reshape(64, -1),
            k3[:, 1::2, :].reshape(64, -1),
        ],
        axis=0,
    )  # [128, Ld/2]
    v_part = vv.reshape(nkt, 128, 65).transpose(1, 0, 2).reshape(128, nkt * 65)
    return np.concatenate([q_part, k_part, v_part], axis=1)


def make_in_maps(query_states, key_states, value_states, Wo):
    q = np.asarray(query_states, dtype=np.float32)
    k = np.asarray(key_states, dtype=np.float32)
    v = np.asarray(value_states, dtype=np.float32)
    Wo = np.asarray(Wo, dtype=np.float32)

    in_maps = []
    for c in range(NCORES):
        b, hs = c // 2, (c % 2) * PB
        blob = np.empty((PB, 128, WSUM), BF16_NP)
        wot = np.empty((PB // 2, 128, HIDDEN), BF16_NP)
        for j in range(PB):
            h = hs + j
            for di, d in enumerate(DILS):
                Ld = LDS[di]
                vv = np.empty((Ld, 65), np.float32)
                vv[:, 0:64] = v[b, h, ::d, :]
                vv[:, 64] = 1.0
                blob[j, :, BOFFS[di] : BOFFS[di] + WS[di]] = _branch_blob(
                    np.ascontiguousarray(q[b, h, ::d, :].T),
                    np.ascontiguousarray(k[b, h, ::d, :].T),
                    vv,
                    di,
                )
            # head-pair-stacked o_proj weights (K=128 contraction)
            wot[j // 2, (j % 2) * 64 : (j % 2 + 1) * 64, :] = Wo[
                :, h * 64 : (h + 1) * 64
            ].T
        in_maps.append({"blob": blob, "wot": wot})
    return in_maps


def combine_outputs(results, bo):
    bo = np.asarray(bo, dtype=np.float32)
    out = np.empty((B, L, HIDDEN), np.float32)
    for b in range(B):
        out[b] = results[2 * b]["out"] + results[2 * b + 1]["out"] + bo
    return out


def kernel(
    query_states,
    key_states,
    value_states,
    Wo,
    bo,
    _trace=False,
    _tmpdir=None,
    _results=[None],
):
    from concourse.bass_utils import run_bass_kernel_spmd

    nc = get_program()
    in_maps = make_in_maps(query_states, key_states, value_states, Wo)
    res = run_bass_kernel_spmd(
        nc, in_maps, list(range(NCORES)), trace=_trace, tmpdir=_tmpdir
    )
    _results[0] = res
    return combine_outputs(res.results, bo)

